# revision 29
# baseline (speedup 1.0000x reference)
"""AdditiveRelationalGraphConvolution on 8 TRN2 NeuronCores.

out = relu(mean_s(features[neighbors]) @ W.T + mean_s(RWT[relations]))

Data-parallel over batch (4096 rows/core); feature table replicated (bf16).
The kernel is SDMA-drain bound on random 512B gathers, so the design
minimizes gather descriptor count and keeps all 4 SWDGE queues loaded:
  - neighbor rows are fetched with dma_gather (int16 indices) from 4 static
    windows of <=32768 rows. Per (group-of-4-tiles, window) the 4 tiles'
    sorted index lists are CONCATENATED COMPACTLY (per-core real counts, no
    per-tile quota padding); the call length N is the max real count over
    the 8 cores rounded to 128 (pad = dummy idx 0, masked by owner=255).
    Each big-window call is split into three sub-gathers on separate tiles
    rotated over the 4 SWDGE queues. Per-slot owner tags (batch row or 255)
    let the device rebuild one-hot matrices (DVE is_equal) and aggregate
    with PE matmuls over each tile's STATIC chunk range (union of the
    per-core slot ranges; chunks straddling two tiles are scanned by both
    with complementary owner masks).
  - relations need NO gather: the 238-row relation table lives in SBUF and
    the host ships a per-tile count matrix cnt[r,b] = #occurrences/16; two
    PE matmuls accumulate cnt.T @ RWT straight into the output PSUM.
  - main transform: psum[b,o] = aggT.T @ (W.T/16) + cnt-term, relu on ACT,
    store bf16 (host upcasts to f32).
"""

import sys

sys.path.insert(0, "/opt/trn_rl_repo")

import numpy as np

N_CORES = 8
B = 32768
S = 16
D = 256
NUM_NODES = 100000
NUM_REL = 238
B_LOC = B // N_CORES  # 4096
P = 128
TILES = B_LOC // P  # 32
GRP = 4  # tiles per gather group
NGRP = TILES // GRP

WIN = [(0, 32768), (32768, 65536), (65536, 98304), (98304, 100000)]
DEAD = 255.0

_CACHE = {}


def _struct_for(neighbors):
    """Static gather structure from the actual input (compile-per-input):
    per (group, window): call length N (max over cores of the group's real
    count, rounded up to 128) and its sub-call split; per (tile, window):
    the static chunk range covering every core's slot range."""
    nb = np.ascontiguousarray(neighbors, dtype=np.int64).reshape(
        N_CORES, TILES, P * S
    )
    counts = np.zeros((N_CORES, TILES, 4), dtype=np.int64)
    for k in range(4):
        counts[:, :, k] = ((nb >= WIN[k][0]) & (nb < WIN[k][1])).sum(axis=2)

    Ncall = np.zeros((NGRP, 4), dtype=np.int64)  # padded call lengths
    sub = {}  # (tg, k) -> list of sub-call lengths (128-multiples)
    crange = np.zeros((TILES, 4, 2), dtype=np.int64)  # static chunk ranges
    for tg in range(NGRP):
        t0 = tg * GRP
        for k in range(4):
            gc = counts[:, t0 : t0 + GRP, k]  # [cores, GRP]
            n = int(gc.sum(axis=1).max())
            N = -(-max(n, 1) // P) * P
            Ncall[tg, k] = N
            if k < 3:
                a = (N // 3) // P * P
                a = max(a, P)
                sub[(tg, k)] = [a, a, N - 2 * a]
            else:
                sub[(tg, k)] = [N]
            start = np.concatenate(
                [np.zeros((N_CORES, 1), dtype=np.int64), gc.cumsum(axis=1)],
                axis=1,
            )
            for ti in range(GRP):
                lo = int(start[:, ti].min()) // P
                hi = -(-int(start[:, ti + 1].max()) // P)
                hi = min(max(hi, lo + 1), N // P)
                crange[t0 + ti, k] = (lo, hi)
    # owner column offset per tile (static layout)
    ncols = (crange[:, :, 1] - crange[:, :, 0]).sum(axis=1)
    coff = np.concatenate([[0], ncols.cumsum()])
    key = (
        tuple(Ncall.ravel().tolist()),
        tuple((k, tuple(v)) for k, v in sorted(sub.items())),
        tuple(crange.ravel().tolist()),
    )
    return {
        "Ncall": Ncall,
        "sub": sub,
        "crange": crange,
        "coff": coff,
        "key": key,
    }


# test.py compatibility: it calls _quotas_for and passes the result around.
_quotas_for = _struct_for


def _build(ST):
    import concourse.tile as tile
    from concourse import bacc, mybir

    Ncall = ST["Ncall"]
    sub = ST["sub"]
    crange = ST["crange"]
    coff = ST["coff"]
    IDXCOLS_G = [int(Ncall[tg].sum()) // 16 for tg in range(NGRP)]
    IDXOFF_G = np.concatenate([[0], np.cumsum(IDXCOLS_G)])
    OWCOLS = int(coff[-1])

    f32 = mybir.dt.float32
    bf16 = mybir.dt.bfloat16
    i16 = mybir.dt.int16

    nc = bacc.Bacc(
        "TRN2",
        target_bir_lowering=False,
        debug=False,
        enable_asserts=False,
        num_devices=N_CORES,
        num_swdge_queues=4,
        dynamic_dma_scratch_size=49152,
    )
    feat = nc.dram_tensor("feat", [NUM_NODES, D], bf16, kind="ExternalInput").ap()
    rwt = nc.dram_tensor("rwt", [2 * P, D], bf16, kind="ExternalInput").ap()
    wT = nc.dram_tensor("wT", [D, D], bf16, kind="ExternalInput").ap()
    nidx = nc.dram_tensor(
        "nidx", [P, int(IDXOFF_G[-1])], i16, kind="ExternalInput"
    ).ap()
    owner = nc.dram_tensor("owner", [P, OWCOLS], bf16, kind="ExternalInput").ap()
    iota = nc.dram_tensor("iota", [P, P], bf16, kind="ExternalInput").ap()
    cnt = nc.dram_tensor("cnt", [P, TILES * 2 * P], bf16, kind="ExternalInput").ap()
    out = nc.dram_tensor("out", [B_LOC, D], bf16, kind="ExternalOutput").ap()

    with tile.TileContext(nc) as tc:
        with (
            tc.tile_pool(name="const", bufs=1) as cp,
            tc.tile_pool(name="gfix", bufs=2) as gfix,
            tc.tile_pool(name="sel", bufs=2) as selp,
            tc.tile_pool(name="small", bufs=3) as small,
            tc.tile_pool(name="psA", bufs=2, space="PSUM") as psA,
            tc.tile_pool(name="psB", bufs=4, space="PSUM") as psB,
        ):
            nidx_sb = cp.tile([P, int(IDXOFF_G[-1])], i16)
            nc.sync.dma_start(out=nidx_sb[:], in_=nidx[:])
            owner_sb = cp.tile([P, OWCOLS], bf16)
            nc.sync.dma_start(out=owner_sb[:], in_=owner[:])
            iota3_sb = cp.tile([P, P], bf16)
            wt_sb = cp.tile([P, 2 * D], bf16)
            rwt_sb = cp.tile([P, 2 * D], bf16)
            cnt_sb = cp.tile([P, TILES * 2 * P], bf16)

            def _late_consts():
                nc.sync.dma_start(out=iota3_sb[:], in_=iota[:])
                nc.sync.dma_start(out=wt_sb[:, 0:D], in_=wT[0:P, :])
                nc.sync.dma_start(out=wt_sb[:, D : 2 * D], in_=wT[P : 2 * P, :])
                nc.sync.dma_start(out=rwt_sb[:, 0:D], in_=rwt[0:P, :])
                nc.sync.dma_start(out=rwt_sb[:, D : 2 * D], in_=rwt[P : 2 * P, :])
                nc.sync.dma_start(out=cnt_sb[:], in_=cnt[:])

            qctr = [0]

            def nextq():
                q = qctr[0] % 4
                qctr[0] += 1
                return q

            MAXC = int((crange[:, :, 1] - crange[:, :, 0]).sum(axis=1).max())
            Gg = {}
            for tg in range(NGRP):
                col0 = int(IDXOFF_G[tg])
                off = 0
                G = {}  # (k, subchunk_base) -> (tile, local chunk count)
                for k in range(4):
                    cbase = 0
                    for si, n in enumerate(sub[(tg, k)]):
                        nch = n // P
                        g = gfix.tile(
                            [P, nch * D],
                            bf16,
                            name=f"g{tg}_{k}_{si}",
                            tag=f"gath{k}{si}",
                            bufs=4 if si < 2 else 3,
                        )
                        w = n // 16
                        nc.gpsimd.dma_gather(
                            out_ap=g[:].rearrange("p (c d) -> p c d", d=D),
                            in_ap=feat[WIN[k][0] : WIN[k][1], :],
                            idxs_ap=nidx_sb[:, col0 + off : col0 + off + w],
                            num_idxs=n,
                            num_idxs_reg=n,
                            elem_size=D,
                            single_packet=False,
                            queue_num=nextq(),
                        )
                        G[(k, cbase)] = (g, nch)
                        cbase += nch
                        off += w
                Gg[tg] = G
                if tg == 0:
                    _late_consts()

                def chunk_slice(k, c):
                    # continuum chunk c of window k -> sub-tile slice
                    for (kk, cb), (g, nch) in Gg[tg].items():
                        if kk == k and cb <= c < cb + nch:
                            lc = c - cb
                            return g[:, lc * D : (lc + 1) * D]
                    raise KeyError((k, c))

                for t in range(tg * GRP, (tg + 1) * GRP):
                    ncols_t = int(
                        (crange[t, :, 1] - crange[t, :, 0]).sum()
                    )
                    sel = selp.tile([P, MAXC * P], bf16, tag="sel")
                    ow = owner_sb[:, int(coff[t]) : int(coff[t]) + ncols_t]
                    nc.vector.tensor_tensor(
                        out=sel[:, : ncols_t * P].rearrange(
                            "p (c b) -> p c b", b=P
                        ),
                        in0=ow[:, :, None].to_broadcast([P, ncols_t, P]),
                        in1=iota3_sb[:, None, :].to_broadcast([P, ncols_t, P]),
                        op=mybir.AluOpType.is_equal,
                    )

                    agT0 = psA.tile([P, P], f32, tag="agT0", space="PSUM")
                    agT1 = psA.tile([P, P], f32, tag="agT1", space="PSUM")
                    ci = 0
                    for k in range(4):
                        for c in range(int(crange[t, k, 0]), int(crange[t, k, 1])):
                            lhs = chunk_slice(k, c)
                            for ic, agT in enumerate((agT0, agT1)):
                                nc.tensor.matmul(
                                    out=agT[:],
                                    lhsT=lhs[:, ic * P : (ic + 1) * P],
                                    rhs=sel[:, ci * P : (ci + 1) * P],
                                    start=(ci == 0),
                                    stop=(ci == ncols_t - 1),
                                )
                            ci += 1
                    aggT = small.tile([P, 2 * P], bf16, tag="aggT")
                    nc.vector.tensor_copy(out=aggT[:, 0:P], in_=agT0[:])
                    nc.vector.tensor_copy(out=aggT[:, P : 2 * P], in_=agT1[:])

                    pm = psB.tile([P, D], f32, tag="pm", space="PSUM")
                    nc.tensor.matmul(
                        out=pm[:],
                        lhsT=aggT[:, 0:P],
                        rhs=wt_sb[:, 0:D],
                        start=True,
                        stop=False,
                    )
                    nc.tensor.matmul(
                        out=pm[:],
                        lhsT=aggT[:, P : 2 * P],
                        rhs=wt_sb[:, D : 2 * D],
                        start=False,
                        stop=False,
                    )
                    for c in range(2):
                        nc.tensor.matmul(
                            out=pm[:],
                            lhsT=cnt_sb[:, (t * 2 + c) * P : (t * 2 + c + 1) * P],
                            rhs=rwt_sb[:, c * D : (c + 1) * D],
                            start=False,
                            stop=(c == 1),
                        )
                    osb = small.tile([P, D], bf16, tag="osb")
                    nc.scalar.activation(
                        out=osb[:], in_=pm[:], func=mybir.ActivationFunctionType.Relu
                    )
                    nc.sync.dma_start(out=out[t * P : (t + 1) * P, :], in_=osb[:])
    nc.compile()
    return nc


def _get_nc(ST):
    key = ("nc", ST["key"])
    if key not in _CACHE:
        _CACHE[key] = _build(ST)
    return _CACHE[key]


def _wrap16(lst, width):
    """Wrap a flat ALL-VALID index list of length width*16 into [128, width]
    int16 (16-partition wrap, replicated to all 8 gpsimd core groups)."""
    n = len(lst)
    assert n == width * 16
    outw = np.asarray(lst, dtype=np.int16).reshape(width, 16).T
    return np.tile(outw, (8, 1))


def _prep_inputs(neighbors, relations, features, weight, relation_weight, QUOTA):
    import ml_dtypes

    ST = QUOTA
    Ncall = ST["Ncall"]
    crange = ST["crange"]
    coff = ST["coff"]
    IDXCOLS_G = [int(Ncall[tg].sum()) // 16 for tg in range(NGRP)]
    IDXOFF_G = np.concatenate([[0], np.cumsum(IDXCOLS_G)])
    OWCOLS = int(coff[-1])

    bf16 = ml_dtypes.bfloat16
    inv_s = np.float32(1.0 / S)

    nb = np.ascontiguousarray(neighbors, dtype=np.int64).reshape(N_CORES, TILES, P, S)
    rl = np.ascontiguousarray(relations, dtype=np.int64).reshape(N_CORES, TILES, P, S)
    feat = np.ascontiguousarray(features.astype(bf16))
    rwt_full = np.zeros((2 * P, D), dtype=np.float32)
    rwt_full[:NUM_REL] = relation_weight.T.astype(np.float32)
    rwt = np.ascontiguousarray(rwt_full.astype(bf16))
    wT = np.ascontiguousarray((weight.T.astype(np.float32) * inv_s).astype(bf16))
    iota = np.ascontiguousarray(
        np.broadcast_to(np.arange(P, dtype=np.float32), (P, P)).astype(bf16)
    )

    in_maps = []
    for core in range(N_CORES):
        nidx = np.zeros((P, int(IDXOFF_G[-1])), dtype=np.int16)
        owner = np.full((P, OWCOLS), DEAD, dtype=np.float32)
        cnt = np.zeros((P, TILES * 2 * P), dtype=np.float32)
        for tg in range(NGRP):
            col = int(IDXOFF_G[tg])
            for k in range(4):
                N = int(Ncall[tg, k])
                # concatenated compact per-tile sorted lists + dummy pad
                lists = []
                owners = []  # (tile_local, slot_range_start, owner_array)
                pos = 0
                tstart = []
                for ti in range(GRP):
                    t = tg * GRP + ti
                    idxs = nb[core, t].ravel()
                    m = (idxs >= WIN[k][0]) & (idxs < WIN[k][1])
                    li = idxs[m] - WIN[k][0]
                    lo = np.repeat(np.arange(P), S)[m]
                    order = np.argsort(li, kind="stable")
                    lists.append(li[order].astype(np.int16))
                    owners.append(lo[order].astype(np.float32))
                    tstart.append(pos)
                    pos += len(li)
                tstart.append(pos)
                assert pos <= N, f"call overflow {pos} > {N}"
                flat = np.concatenate(
                    lists + [np.zeros(N - pos, dtype=np.int16)]
                )
                w = N // 16
                nidx[:, col : col + w] = _wrap16(flat, w)
                col += w
                # owner slots per (tile, static chunk range)
                slotown = np.full(N, DEAD, dtype=np.float32)
                # (filled per tile below from owners; a slot belongs to
                # exactly one tile for this core)
                for ti in range(GRP):
                    s0, s1 = tstart[ti], tstart[ti + 1]
                    slotown[s0:s1] = owners[ti]
                ocol = int(coff[tg * GRP])
                for ti in range(GRP):
                    t = tg * GRP + ti
                    # column offset of this tile's window-k chunk block
                    oc = int(coff[t])
                    for kk in range(k):
                        oc += int(crange[t, kk, 1] - crange[t, kk, 0])
                    s0, s1 = tstart[ti], tstart[ti + 1]
                    for j, c in enumerate(
                        range(int(crange[t, k, 0]), int(crange[t, k, 1]))
                    ):
                        cw = np.full(P, DEAD, dtype=np.float32)
                        lo_s = max(c * P, s0)
                        hi_s = min((c + 1) * P, s1)
                        if hi_s > lo_s:
                            cw[lo_s - c * P : hi_s - c * P] = slotown[lo_s:hi_s]
                        owner[:, oc + j] = cw
            # relation count matrix
            for ti in range(GRP):
                t = tg * GRP + ti
                rt = rl[core, t]
                counts = (
                    np.bincount(
                        rt.ravel() * P + np.repeat(np.arange(P), S),
                        minlength=2 * P * P,
                    )
                    .reshape(2 * P, P)
                    .astype(np.float32)
                )
                cnt[:, t * 2 * P : t * 2 * P + P] = counts[:P] * inv_s
                cnt[:, t * 2 * P + P : (t + 1) * 2 * P] = counts[P:] * inv_s
        in_maps.append(
            {
                "feat": feat,
                "rwt": rwt,
                "wT": wT,
                "nidx": nidx,
                "owner": owner.astype(bf16),
                "iota": iota,
                "cnt": cnt.astype(bf16),
            }
        )
    return in_maps


def run(in_maps, QUOTA, trace=False, tmpdir=None):
    from concourse.bass_utils import run_bass_kernel_spmd

    nc = _get_nc(QUOTA)
    res = run_bass_kernel_spmd(
        nc, in_maps, core_ids=list(range(N_CORES)), trace=trace, tmpdir=tmpdir
    )
    out = np.concatenate([res.results[i]["out"] for i in range(N_CORES)], axis=0)
    return out.astype(np.float32), res


def kernel(neighbors, relations, features, weight, relation_weight):
    ST = _struct_for(neighbors)
    in_maps = _prep_inputs(
        neighbors, relations, features, weight, relation_weight, ST
    )
    out, _ = run(in_maps, ST, trace=False)
    return out


# revision 30
# speedup vs baseline: 1.1815x; 1.1815x over previous
"""AdditiveRelationalGraphConvolution on 8 TRN2 NeuronCores.

out = relu(mean_s(features[neighbors]) @ W.T + mean_s(RWT[relations]))

Data-parallel over batch (4096 rows/core); feature table replicated (bf16).
The kernel is SDMA-drain bound on random 512B gathers, so the design
minimizes gather descriptor count and keeps all 4 SWDGE queues loaded:
  - neighbor rows are fetched with dma_gather (int16 indices) from 4 static
    windows of <=32768 rows. Per (group-of-4-tiles, window) the 4 tiles'
    sorted index lists are CONCATENATED COMPACTLY (per-core real counts, no
    per-tile quota padding); the call length N is the max real count over
    the 8 cores rounded to 128 (pad = dummy idx 0, masked by owner=255).
    Each big-window call is split into three sub-gathers on separate tiles
    rotated over the 4 SWDGE queues. Per-slot owner tags (batch row or 255)
    let the device rebuild one-hot matrices (DVE is_equal) and aggregate
    with PE matmuls over each tile's STATIC chunk range (union of the
    per-core slot ranges; chunks straddling two tiles are scanned by both
    with complementary owner masks).
  - relations need NO gather: the 238-row relation table lives in SBUF and
    the host ships a per-tile count matrix cnt[r,b] = #occurrences/16; two
    PE matmuls accumulate cnt.T @ RWT straight into the output PSUM.
  - main transform: psum[b,o] = aggT.T @ (W.T/16) + cnt-term, relu on ACT,
    store bf16 (host upcasts to f32).
"""

import sys

sys.path.insert(0, "/opt/trn_rl_repo")

import numpy as np

N_CORES = 8
B = 32768
S = 16
D = 256
NUM_NODES = 100000
NUM_REL = 238
B_LOC = B // N_CORES  # 4096
P = 128
TILES = B_LOC // P  # 32
GRP = 4  # tiles per gather group
NGRP = TILES // GRP

WIN = [(0, 32768), (32768, 65536), (65536, 98304), (98304, 100000)]
DEAD = 255.0

_CACHE = {}


def _struct_for(neighbors):
    """Static gather structure from the actual input (compile-per-input):
    per (group, window): call length N (max over cores of the group's real
    count, rounded up to 128) and its sub-call split; per (tile, window):
    the static chunk range covering every core's slot range."""
    nb = np.ascontiguousarray(neighbors, dtype=np.int64).reshape(
        N_CORES, TILES, P * S
    )
    counts = np.zeros((N_CORES, TILES, 4), dtype=np.int64)
    for k in range(4):
        counts[:, :, k] = ((nb >= WIN[k][0]) & (nb < WIN[k][1])).sum(axis=2)

    Ncall = np.zeros((NGRP, 4), dtype=np.int64)  # padded call lengths
    sub = {}  # (tg, k) -> list of sub-call lengths (128-multiples)
    crange = np.zeros((TILES, 4, 2), dtype=np.int64)  # static chunk ranges
    for tg in range(NGRP):
        t0 = tg * GRP
        for k in range(4):
            gc = counts[:, t0 : t0 + GRP, k]  # [cores, GRP]
            n = int(gc.sum(axis=1).max())
            N = -(-max(n, 1) // P) * P
            Ncall[tg, k] = N
            if k < 3:
                a = (N // 3) // P * P
                a = max(a, P)
                sub[(tg, k)] = [a, a, N - 2 * a]
            else:
                sub[(tg, k)] = [N]
            start = np.concatenate(
                [np.zeros((N_CORES, 1), dtype=np.int64), gc.cumsum(axis=1)],
                axis=1,
            )
            for ti in range(GRP):
                lo = int(start[:, ti].min()) // P
                hi = -(-int(start[:, ti + 1].max()) // P)
                hi = min(max(hi, lo + 1), N // P)
                crange[t0 + ti, k] = (lo, hi)
    # owner column offset per tile (static layout)
    ncols = (crange[:, :, 1] - crange[:, :, 0]).sum(axis=1)
    coff = np.concatenate([[0], ncols.cumsum()])
    key = (
        tuple(Ncall.ravel().tolist()),
        tuple((k, tuple(v)) for k, v in sorted(sub.items())),
        tuple(crange.ravel().tolist()),
    )
    return {
        "Ncall": Ncall,
        "sub": sub,
        "crange": crange,
        "coff": coff,
        "key": key,
    }


# test.py compatibility: it calls _quotas_for and passes the result around.
_quotas_for = _struct_for


def _build(ST):
    import concourse.tile as tile
    from concourse import bacc, mybir

    Ncall = ST["Ncall"]
    sub = ST["sub"]
    crange = ST["crange"]
    coff = ST["coff"]
    IDXCOLS_G = [int(Ncall[tg].sum()) // 16 for tg in range(NGRP)]
    IDXOFF_G = np.concatenate([[0], np.cumsum(IDXCOLS_G)])
    OWCOLS = int(coff[-1])

    f32 = mybir.dt.float32
    bf16 = mybir.dt.bfloat16
    i16 = mybir.dt.int16

    nc = bacc.Bacc(
        "TRN2",
        target_bir_lowering=False,
        debug=False,
        enable_asserts=False,
        num_devices=N_CORES,
        num_swdge_queues=4,
        dynamic_dma_scratch_size=49152,
    )
    feat = nc.dram_tensor("feat", [NUM_NODES, D], bf16, kind="ExternalInput").ap()
    rwt = nc.dram_tensor("rwt", [2 * P, D], bf16, kind="ExternalInput").ap()
    wT = nc.dram_tensor("wT", [D, D], bf16, kind="ExternalInput").ap()
    nidx = nc.dram_tensor(
        "nidx", [P, int(IDXOFF_G[-1])], i16, kind="ExternalInput"
    ).ap()
    owner = nc.dram_tensor("owner", [P, OWCOLS], bf16, kind="ExternalInput").ap()
    iota = nc.dram_tensor("iota", [P, P], bf16, kind="ExternalInput").ap()
    cnt = nc.dram_tensor("cnt", [P, TILES * 2 * P], bf16, kind="ExternalInput").ap()
    out = nc.dram_tensor("out", [B_LOC, D], bf16, kind="ExternalOutput").ap()

    with tile.TileContext(nc) as tc:
        with (
            tc.tile_pool(name="const", bufs=1) as cp,
            tc.tile_pool(name="gfix", bufs=2) as gfix,
            tc.tile_pool(name="sel", bufs=2) as selp,
            tc.tile_pool(name="small", bufs=3) as small,
            tc.tile_pool(name="psA", bufs=2, space="PSUM") as psA,
            tc.tile_pool(name="psB", bufs=4, space="PSUM") as psB,
        ):
            nidx_sb = cp.tile([P, int(IDXOFF_G[-1])], i16)
            nc.sync.dma_start(out=nidx_sb[:], in_=nidx[:])
            owner_sb = cp.tile([P, OWCOLS], bf16)
            nc.sync.dma_start(out=owner_sb[:], in_=owner[:])
            iota3_sb = cp.tile([P, P], bf16)
            nc.sync.dma_start(out=iota3_sb[:], in_=iota[:])
            wt_sb = cp.tile([P, 2 * D], bf16)
            nc.sync.dma_start(out=wt_sb[:, 0:D], in_=wT[0:P, :])
            nc.sync.dma_start(out=wt_sb[:, D : 2 * D], in_=wT[P : 2 * P, :])
            rwt_sb = cp.tile([P, 2 * D], bf16)
            nc.sync.dma_start(out=rwt_sb[:, 0:D], in_=rwt[0:P, :])
            nc.sync.dma_start(out=rwt_sb[:, D : 2 * D], in_=rwt[P : 2 * P, :])
            cnt_sb = cp.tile([P, TILES * 2 * P], bf16)
            nc.sync.dma_start(out=cnt_sb[:], in_=cnt[:])

            qctr = [0]

            def nextq():
                q = qctr[0] % 4
                qctr[0] += 1
                return q

            MAXC = int((crange[:, :, 1] - crange[:, :, 0]).sum(axis=1).max())
            Gg = {}
            for tg in range(NGRP):
                col0 = int(IDXOFF_G[tg])
                off = 0
                G = {}  # (k, subchunk_base) -> (tile, local chunk count)
                for k in range(4):
                    cbase = 0
                    for si, n in enumerate(sub[(tg, k)]):
                        nch = n // P
                        g = gfix.tile(
                            [P, nch * D],
                            bf16,
                            name=f"g{tg}_{k}_{si}",
                            tag=f"gath{k}{si}",
                            bufs=4 if si < 2 else 3,
                        )
                        w = n // 16
                        nc.gpsimd.dma_gather(
                            out_ap=g[:].rearrange("p (c d) -> p c d", d=D),
                            in_ap=feat[WIN[k][0] : WIN[k][1], :],
                            idxs_ap=nidx_sb[:, col0 + off : col0 + off + w],
                            num_idxs=n,
                            num_idxs_reg=n,
                            elem_size=D,
                            single_packet=False,
                            queue_num=nextq(),
                        )
                        G[(k, cbase)] = (g, nch)
                        cbase += nch
                        off += w
                Gg[tg] = G

                def chunk_slice(k, c):
                    # continuum chunk c of window k -> sub-tile slice
                    for (kk, cb), (g, nch) in Gg[tg].items():
                        if kk == k and cb <= c < cb + nch:
                            lc = c - cb
                            return g[:, lc * D : (lc + 1) * D]
                    raise KeyError((k, c))

                for t in range(tg * GRP, (tg + 1) * GRP):
                    ncols_t = int(
                        (crange[t, :, 1] - crange[t, :, 0]).sum()
                    )
                    sel = selp.tile([P, MAXC * P], bf16, tag="sel")
                    ow = owner_sb[:, int(coff[t]) : int(coff[t]) + ncols_t]
                    nc.vector.tensor_tensor(
                        out=sel[:, : ncols_t * P].rearrange(
                            "p (c b) -> p c b", b=P
                        ),
                        in0=ow[:, :, None].to_broadcast([P, ncols_t, P]),
                        in1=iota3_sb[:, None, :].to_broadcast([P, ncols_t, P]),
                        op=mybir.AluOpType.is_equal,
                    )

                    agT0 = psA.tile([P, P], f32, tag="agT0", space="PSUM")
                    agT1 = psA.tile([P, P], f32, tag="agT1", space="PSUM")
                    ci = 0
                    for k in range(4):
                        for c in range(int(crange[t, k, 0]), int(crange[t, k, 1])):
                            lhs = chunk_slice(k, c)
                            for ic, agT in enumerate((agT0, agT1)):
                                nc.tensor.matmul(
                                    out=agT[:],
                                    lhsT=lhs[:, ic * P : (ic + 1) * P],
                                    rhs=sel[:, ci * P : (ci + 1) * P],
                                    start=(ci == 0),
                                    stop=(ci == ncols_t - 1),
                                )
                            ci += 1
                    aggT = small.tile([P, 2 * P], bf16, tag="aggT")
                    nc.vector.tensor_copy(out=aggT[:, 0:P], in_=agT0[:])
                    nc.vector.tensor_copy(out=aggT[:, P : 2 * P], in_=agT1[:])

                    pm = psB.tile([P, D], f32, tag="pm", space="PSUM")
                    nc.tensor.matmul(
                        out=pm[:],
                        lhsT=aggT[:, 0:P],
                        rhs=wt_sb[:, 0:D],
                        start=True,
                        stop=False,
                    )
                    nc.tensor.matmul(
                        out=pm[:],
                        lhsT=aggT[:, P : 2 * P],
                        rhs=wt_sb[:, D : 2 * D],
                        start=False,
                        stop=False,
                    )
                    for c in range(2):
                        nc.tensor.matmul(
                            out=pm[:],
                            lhsT=cnt_sb[:, (t * 2 + c) * P : (t * 2 + c + 1) * P],
                            rhs=rwt_sb[:, c * D : (c + 1) * D],
                            start=False,
                            stop=(c == 1),
                        )
                    osb = small.tile([P, D], bf16, tag="osb")
                    nc.scalar.activation(
                        out=osb[:], in_=pm[:], func=mybir.ActivationFunctionType.Relu
                    )
                    nc.sync.dma_start(out=out[t * P : (t + 1) * P, :], in_=osb[:])
    nc.compile()
    return nc


def _get_nc(ST):
    key = ("nc", ST["key"])
    if key not in _CACHE:
        _CACHE[key] = _build(ST)
    return _CACHE[key]


def _wrap16(lst, width):
    """Wrap a flat ALL-VALID index list of length width*16 into [128, width]
    int16 (16-partition wrap, replicated to all 8 gpsimd core groups)."""
    n = len(lst)
    assert n == width * 16
    outw = np.asarray(lst, dtype=np.int16).reshape(width, 16).T
    return np.tile(outw, (8, 1))


def _prep_inputs(neighbors, relations, features, weight, relation_weight, QUOTA):
    import ml_dtypes

    ST = QUOTA
    Ncall = ST["Ncall"]
    crange = ST["crange"]
    coff = ST["coff"]
    IDXCOLS_G = [int(Ncall[tg].sum()) // 16 for tg in range(NGRP)]
    IDXOFF_G = np.concatenate([[0], np.cumsum(IDXCOLS_G)])
    OWCOLS = int(coff[-1])

    bf16 = ml_dtypes.bfloat16
    inv_s = np.float32(1.0 / S)

    nb = np.ascontiguousarray(neighbors, dtype=np.int64).reshape(N_CORES, TILES, P, S)
    rl = np.ascontiguousarray(relations, dtype=np.int64).reshape(N_CORES, TILES, P, S)
    feat = np.ascontiguousarray(features.astype(bf16))
    rwt_full = np.zeros((2 * P, D), dtype=np.float32)
    rwt_full[:NUM_REL] = relation_weight.T.astype(np.float32)
    rwt = np.ascontiguousarray(rwt_full.astype(bf16))
    wT = np.ascontiguousarray((weight.T.astype(np.float32) * inv_s).astype(bf16))
    iota = np.ascontiguousarray(
        np.broadcast_to(np.arange(P, dtype=np.float32), (P, P)).astype(bf16)
    )

    in_maps = []
    for core in range(N_CORES):
        nidx = np.zeros((P, int(IDXOFF_G[-1])), dtype=np.int16)
        owner = np.full((P, OWCOLS), DEAD, dtype=np.float32)
        cnt = np.zeros((P, TILES * 2 * P), dtype=np.float32)
        for tg in range(NGRP):
            col = int(IDXOFF_G[tg])
            for k in range(4):
                N = int(Ncall[tg, k])
                # concatenated compact per-tile sorted lists + dummy pad
                lists = []
                owners = []  # (tile_local, slot_range_start, owner_array)
                pos = 0
                tstart = []
                for ti in range(GRP):
                    t = tg * GRP + ti
                    idxs = nb[core, t].ravel()
                    m = (idxs >= WIN[k][0]) & (idxs < WIN[k][1])
                    li = idxs[m] - WIN[k][0]
                    lo = np.repeat(np.arange(P), S)[m]
                    order = np.argsort(li, kind="stable")
                    lists.append(li[order].astype(np.int16))
                    owners.append(lo[order].astype(np.float32))
                    tstart.append(pos)
                    pos += len(li)
                tstart.append(pos)
                assert pos <= N, f"call overflow {pos} > {N}"
                flat = np.concatenate(
                    lists + [np.zeros(N - pos, dtype=np.int16)]
                )
                w = N // 16
                nidx[:, col : col + w] = _wrap16(flat, w)
                col += w
                # owner slots per (tile, static chunk range)
                slotown = np.full(N, DEAD, dtype=np.float32)
                # (filled per tile below from owners; a slot belongs to
                # exactly one tile for this core)
                for ti in range(GRP):
                    s0, s1 = tstart[ti], tstart[ti + 1]
                    slotown[s0:s1] = owners[ti]
                ocol = int(coff[tg * GRP])
                for ti in range(GRP):
                    t = tg * GRP + ti
                    # column offset of this tile's window-k chunk block
                    oc = int(coff[t])
                    for kk in range(k):
                        oc += int(crange[t, kk, 1] - crange[t, kk, 0])
                    s0, s1 = tstart[ti], tstart[ti + 1]
                    for j, c in enumerate(
                        range(int(crange[t, k, 0]), int(crange[t, k, 1]))
                    ):
                        cw = np.full(P, DEAD, dtype=np.float32)
                        lo_s = max(c * P, s0)
                        hi_s = min((c + 1) * P, s1)
                        if hi_s > lo_s:
                            cw[lo_s - c * P : hi_s - c * P] = slotown[lo_s:hi_s]
                        owner[:, oc + j] = cw
            # relation count matrix
            for ti in range(GRP):
                t = tg * GRP + ti
                rt = rl[core, t]
                counts = (
                    np.bincount(
                        rt.ravel() * P + np.repeat(np.arange(P), S),
                        minlength=2 * P * P,
                    )
                    .reshape(2 * P, P)
                    .astype(np.float32)
                )
                cnt[:, t * 2 * P : t * 2 * P + P] = counts[:P] * inv_s
                cnt[:, t * 2 * P + P : (t + 1) * 2 * P] = counts[P:] * inv_s
        in_maps.append(
            {
                "feat": feat,
                "rwt": rwt,
                "wT": wT,
                "nidx": nidx,
                "owner": owner.astype(bf16),
                "iota": iota,
                "cnt": cnt.astype(bf16),
            }
        )
    return in_maps


def run(in_maps, QUOTA, trace=False, tmpdir=None):
    from concourse.bass_utils import run_bass_kernel_spmd

    nc = _get_nc(QUOTA)
    res = run_bass_kernel_spmd(
        nc, in_maps, core_ids=list(range(N_CORES)), trace=trace, tmpdir=tmpdir
    )
    out = np.concatenate([res.results[i]["out"] for i in range(N_CORES)], axis=0)
    return out.astype(np.float32), res


def kernel(neighbors, relations, features, weight, relation_weight):
    ST = _struct_for(neighbors)
    in_maps = _prep_inputs(
        neighbors, relations, features, weight, relation_weight, ST
    )
    out, _ = run(in_maps, ST, trace=False)
    return out
